# revision 2
# baseline (speedup 1.0000x reference)
"""Trainium2 Bass kernel for nn_Decoder — v2, gather-free.

Architecture (8-core SPMD, one shared program):
  - Gather commutes with the per-row linear map: stages pre-multiply the
    gathered operand into small z-tables (z1 = feats@W1a, z2 = x1@W2a,
    z3 = x2@W3a), kept point-major in SBUF.
  - Upsample expansions are ONE-HOT MATMULS on the tensor engine:
    channel-major psum[ch, cols] += z_pm_rank.T @ P[:, cols], where P is a
    host-built fp16 one-hot matrix (row = source slot % 128) and points are
    sorted so each run of columns draws from a single 128-slot source rank.
    No DMA gathers anywhere.
  - Stage-2 points are laid out class-major (stage-3 fanout class), rank-minor,
    padded per (class, rank) cell to the max across cores so the program is
    shared. Stage-3 expansion is the stride-0 class-broadcast DVE/GPSIMD add.
  - Everything HBM-facing is fp16 (host casts): skips, weights, one-hots,
    output. Matmuls run fp16 (4x over fp32); psum/BN stats stay fp32.
  - BN stats per chunk from PSUM; two tiny AllReduces; BN-affine + LeakyReLU
    fused in one ACT op with per-partition scale/bias.
"""

import sys

sys.path.insert(0, "/opt/trn_rl_repo")

import numpy as np

from concourse import bacc, bass_utils, mybir, tile

dt = mybir.dt
AF = mybir.ActivationFunctionType
OP = mybir.AluOpType
AX = mybir.AxisListType

NCORES = 8
EPS = 1e-5
SLOPE = 0.01

N3, N2, N1, N0 = 4096, 16384, 65536, 262144
C3 = 512  # feats rows per core

S1CMAX = 10240  # stage-3 skip chunk columns resident in SBUF
FCH = 512  # psum chunk width


def _ceil_to(x, m):
    return ((x + m - 1) // m) * m


def _bucket(owner, ncores):
    order = np.argsort(owner, kind="stable")
    counts = np.bincount(owner, minlength=ncores)
    splits = np.split(order, np.cumsum(counts)[:-1])
    return splits, counts


def _cls_layout(M, kmax):
    """M[k] = class-k slot capacity. Returns (src_off, out_off, n2p, n3p,
    s1chunks) where s1chunks groups class pieces into <=S1CMAX output-column
    loads, pieces of <=512 cols each."""
    K = len(M)
    src_off = [0] * K
    o = 0
    for k in range(K):
        src_off[k] = o
        o += M[k]
    n2p = o
    out_off = [0] * K
    o = 0
    for k in range(1, K):
        out_off[k] = o
        o += M[k] * k
    n3p = o

    chunks = []
    cur0, curlen, curpieces = 0, 0, []
    for k in range(1, K):
        blk = M[k] * k
        if blk == 0:
            continue
        if curlen + blk > S1CMAX and curlen > 0:
            chunks.append((cur0, curlen, curpieces))
            cur0, curlen, curpieces = out_off[k], 0, []
        ck_max = (512 // k) * k
        done = 0
        while done < blk:
            ck = min(ck_max, blk - done)
            curpieces.append((k, out_off[k] + done, src_off[k] + done // k, ck))
            done += ck
        curlen += blk
    if curlen > 0:
        chunks.append((cur0, curlen, curpieces))
    return src_off, out_off, n2p, n3p, chunks


def _pair_ops(s1chunks):
    """Pair consecutive same-(k, ck) pieces within each chunk for 2-stack
    processing. Returns (chunk_ops, PK): chunk_ops[i] = list of
    ("pair", k, ck, gA, s0A, po) | ("single", k, ck, g0, s0, po);
    PK = packed out_h column count."""
    chunk_ops = []
    po = 0
    for cols0, clen, pieces in s1chunks:
        ops = []
        i = 0
        while i < len(pieces):
            k, g0, s0, ck = pieces[i]
            if i + 1 < len(pieces):
                k2, g2, s2, ck2 = pieces[i + 1]
                if k2 == k and ck2 == ck and g2 == g0 + ck:
                    ops.append(("pair", k, ck, g0, s0, po))  # gB=g0+ck, s0B=s0+nj
                    po += ck
                    i += 2
                    continue
            ops.append(("single", k, ck, g0, s0, po))
            po += ck
            i += 1
        chunk_ops.append(ops)
    return chunk_ops, po


# ---------------------------------------------------------------------------
# device program
# ---------------------------------------------------------------------------


def _build_program(n1p, B1, n2p, cells2, Mtuple, kmax):
    """B1: tuple of 5 cumulative stage-1 rank boundaries (incl end of real
    slots). cells2: tuple of (rank, start, cap) runs over stage-2 slots,
    class-major order. Mtuple: per-class slot capacities for stage 3."""
    src_off, out_off, n2p_c, n3p, s1chunks = _cls_layout(list(Mtuple), kmax)
    assert n2p_c <= n2p

    nc = bacc.Bacc(
        "TRN2",
        target_bir_lowering=False,
        debug=False,
        num_devices=NCORES,
        num_swdge_queues=4,
    )

    f16 = dt.float16
    f32 = dt.float32

    NZ1R = 4  # feats source ranks
    NZ2R = n1p // 128  # stage-1 slot ranks
    BLOB16 = 2929  # packed fp16 params+feats (see host packing)
    chunk_ops, PK = _pair_ops(s1chunks)
    PWMAX = max(sum(o[2] for o in ops) for ops in chunk_ops)

    # ---- I/O (fp16 unless noted) ----
    s3T_h = nc.dram_tensor("s3T", [512, n1p], f16, kind="ExternalInput")
    s2T_h = nc.dram_tensor("s2T", [257, n2p], f16, kind="ExternalInput")
    s1T_h = nc.dram_tensor("s1T", [128, n3p], f16, kind="ExternalInput")
    P1_h = nc.dram_tensor("P1", [128, n1p], f16, kind="ExternalInput")
    P2_h = nc.dram_tensor("P2", [128, n2p], f16, kind="ExternalInput")
    blob_h = nc.dram_tensor("blob", [128, BLOB16], f16, kind="ExternalInput")
    blobf_h = nc.dram_tensor("blobf", [128, 200], f32, kind="ExternalInput")
    out_h = nc.dram_tensor("out", [98, PK], f16, kind="ExternalOutput")

    def bn_scalars(sb, stats, gbe, n_true, P, name):
        """stats [P,2]=(sum,sumsq) -> (s, t): s=g*rsqrt(var+eps), t=be-mean*s."""
        mean = sb.tile([P, 1], f32, tag=f"{name}_mean")
        ms = sb.tile([P, 1], f32, tag=f"{name}_ms")
        nc.vector.tensor_scalar(mean[:], stats[:, 0:1], 1.0 / n_true, None, OP.mult)
        nc.vector.tensor_scalar(ms[:], stats[:, 1:2], 1.0 / n_true, None, OP.mult)
        var = sb.tile([P, 1], f32, tag=f"{name}_var")
        nc.vector.tensor_tensor(var[:], mean[:], mean[:], OP.mult)
        nc.vector.tensor_tensor(var[:], ms[:], var[:], OP.subtract)
        nc.vector.tensor_scalar(var[:], var[:], EPS, None, OP.add)
        std = sb.tile([P, 1], f32, tag=f"{name}_std")
        nc.scalar.activation(std[:], var[:], AF.Sqrt)
        s = sb.tile([P, 1], f32, tag=f"{name}_s")
        nc.vector.reciprocal(s[:], std[:])
        nc.vector.tensor_tensor(s[:], s[:], gbe[:, 0:1], OP.mult)
        t = sb.tile([P, 1], f32, tag=f"{name}_t")
        nc.vector.tensor_tensor(t[:], mean[:], s[:], OP.mult)
        nc.vector.tensor_tensor(t[:], gbe[:, 1:2], t[:], OP.subtract)
        return s, t

    def chunks_of(n, step=FCH):
        out = []
        o = 0
        while o < n:
            out.append((o, min(o + step, n)))
            o += step
        return out

    def runs_in(bounds, f0, f1):
        """bounds: list of (rank, a, b) slot ranges. Clip to [f0, f1)."""
        out = []
        for r, a, b in bounds:
            a2, b2 = max(a, f0), min(b, f1)
            if a2 < b2:
                out.append((r, a2, b2))
        return out

    bounds1 = [(r, B1[r], B1[r + 1]) for r in range(NZ1R)]
    bounds2 = [(r, a, a + cap) for (r, a, cap) in cells2]

    with tile.TileContext(nc) as tc:
        from contextlib import ExitStack

        octx = ExitStack()
        with octx:
            sb = octx.enter_context(tc.tile_pool(name="persist", bufs=1))
            dram = octx.enter_context(tc.tile_pool(name="dram", bufs=1, space="DRAM"))

            # ---- packed params (one DMA each) ----
            blob = sb.tile([128, BLOB16], f16)
            nc.sync.dma_start(blob[:], blob_h.ap())
            blobf = sb.tile([128, 200], f32)
            nc.sync.dma_start(blobf[:], blobf_h.ap())
            Mhalf = blobf[:, 8:72]      # [128, 64]: out[c] = in[c] + in[c+64]
            MTrep = blobf[0:64, 72:200]  # [64, 128]: replicate [64] -> [128]
            b3stk = blobf[:, 7:8]        # b3 at rows 0-33 and 64-97

            def W1a_k(k):
                return blob[:, k * 129 : (k + 1) * 129]

            W1ax = blob[0:2, 258:387]

            def W1b_k(k):
                return blob[:, 387 + k * 129 : 387 + k * 129 + 128]

            def W1bx_k(k):
                return blob[:, 387 + k * 129 + 128 : 387 + (k + 1) * 129]

            W2a = blob[:, 903:967]
            W2ax = blob[0:1, 967:1031]

            def W2b_k(k):
                return blob[:, 1031 + k * 64 : 1031 + (k + 1) * 64]

            W3a = blob[0:64, 1159:1193]
            W3b = blob[:, 1193:1227]

            def featsT_k(k):
                return blob[:, 1227 + k * 512 : 1227 + (k + 1) * 512]

            featsTx = blob[0:2, 2251:2763]
            W3a2a = blob[0:65, 2763:2797]
            W3a2b = blob[0:65, 2829:2863]
            bn1 = blobf[:, 0:2]
            bn1x = blobf[0:1, 2:4]
            bn2 = blobf[0:64, 4:6]
            b3p = blobf[0:34, 6:7]

            # ---- pools: poolB (stage-2 inputs) outer, poolA (stage-1) inner
            poolB = ExitStack()  # dies after st2
            pB = poolB.enter_context(tc.tile_pool(name="poolB", bufs=1))
            P2 = pB.tile([128, n2p], f16)
            s2T = pB.tile([128, 2, n2p], f16)
            y2T = pB.tile([64, n2p], f16)
            poolA = ExitStack()  # dies after z2
            pA = poolA.enter_context(tc.tile_pool(name="poolA", bufs=1))
            P1 = pA.tile([128, n1p], f16)
            s3T = pA.tile([128, 4, n1p], f16)

            # loads in priority order (stage-1 inputs first)
            nc.sync.dma_start(P1[:], P1_h.ap())
            for k in range(4):
                nc.sync.dma_start(s3T[:, k, :], s3T_h.ap()[k * 128 : (k + 1) * 128, :])
            nc.sync.dma_start(P2[:], P2_h.ap())
            for k in range(2):
                nc.sync.dma_start(s2T[:, k, :], s2T_h.ap()[k * 128 : (k + 1) * 128, :])

            # ---- z tables + cross-stage tiles ----
            z1pm = sb.tile([128, NZ1R, 129], f16)
            z2pm = sb.tile([128, NZ2R, 64], f16)
            x2T = sb.tile([65, n2p], f16)
            nc.sync.dma_start(x2T[64:65, :], s2T_h.ap()[256:257, :])
            y1T = pA.tile([128, n1p], f16)
            y1Tx = pA.tile([1, n1p], f16)
            x1T = pA.tile([128, n1p], f16)
            x1Tx = pA.tile([1, n1p], f16)

            ch1 = chunks_of(n1p)
            ch2 = chunks_of(n2p)
            st1_sum = sb.tile([128, len(ch1)], f32)
            st1_sq = sb.tile([128, len(ch1)], f32)
            st1x_sum = sb.tile([1, len(ch1)], f32)
            st1x_sq = sb.tile([1, len(ch1)], f32)
            st2_sum = sb.tile([64, len(ch2)], f32)
            st2_sq = sb.tile([64, len(ch2)], f32)

            # ---------------- z1 = feats @ W1a (point-major) ----------------
            with (
                nc.named_scope("ph_z1"),
                tc.tile_pool(name="z1ps", bufs=2, space="PSUM") as z1ps,
            ):
                for t in range(NZ1R):
                    ps = z1ps.tile([128, 129], f32, tag="ps")
                    c0 = t * 128
                    nc.tensor.matmul(
                        ps[:], featsT_k(0)[:, c0 : c0 + 128], W1a_k(0),
                        start=True, stop=False,
                    )
                    nc.tensor.matmul(
                        ps[:], featsT_k(1)[:, c0 : c0 + 128], W1a_k(1),
                        start=False, stop=False,
                    )
                    nc.tensor.matmul(
                        ps[:], featsTx[:, c0 : c0 + 128], W1ax,
                        start=False, stop=True,
                    )
                    nc.scalar.activation(z1pm[:, t, :], ps[:], AF.Copy)

            # ---------------- stage 1 (channel-major) ----------------
            with (
                nc.named_scope("ph_st1"),
                tc.tile_pool(name="yps", bufs=2, space="PSUM") as yps,
                tc.tile_pool(name="ypsx", bufs=2, space="PSUM") as ypsx,
                tc.tile_pool(name="sqscr", bufs=2) as sqscr,
            ):
                for ci, (f0, f1) in enumerate(ch1):
                    fl = f1 - f0
                    ps = yps.tile([128, FCH], f32, tag="ps")
                    psx = ypsx.tile([1, FCH], f32, tag="psx")
                    rr = runs_in(bounds1, f0, f1)
                    mms = [(ps, W1b_k(k), s3T[:, k, f0:f1], 0, fl) for k in range(4)]
                    mms += [(ps, z1pm[:, r, 0:128], P1[:, a:b], a - f0, b - f0) for r, a, b in rr]
                    mmx = [(psx, W1bx_k(k), s3T[:, k, f0:f1], 0, fl) for k in range(4)]
                    mmx += [(psx, z1pm[:, r, 128:129], P1[:, a:b], a - f0, b - f0) for r, a, b in rr]
                    for group in (mms, mmx):
                        for mi, (po, lhs, rhs, a, b) in enumerate(group):
                            nc.tensor.matmul(
                                po[:, a:b], lhs, rhs,
                                start=(mi == 0), stop=(mi == len(group) - 1),
                                skip_group_check=True,
                            )
                    # stats + copy out of psum
                    scr = sqscr.tile([128, FCH], f16, tag="scr")
                    scrx = sqscr.tile([1, FCH], f16, tag="scrx")
                    nc.vector.tensor_reduce(
                        st1_sum[:, ci : ci + 1], ps[:, :fl], AX.X, OP.add
                    )
                    nc.scalar.activation(
                        scr[:, :fl], ps[:, :fl], AF.Square,
                        accum_out=st1_sq[:, ci : ci + 1],
                    )
                    nc.scalar.activation(y1T[:, f0:f1], ps[:, :fl], AF.Copy)
                    nc.vector.tensor_reduce(
                        st1x_sum[:, ci : ci + 1], psx[:, :fl], AX.X, OP.add
                    )
                    nc.scalar.activation(
                        scrx[:, :fl], psx[:, :fl], AF.Square,
                        accum_out=st1x_sq[:, ci : ci + 1],
                    )
                    nc.scalar.activation(y1Tx[:, f0:f1], psx[:, :fl], AF.Copy)

                # ---- stage-2 partial (W2b x s2T): fills the AR1 window ----
                with tc.tile_pool(name="yps2a", bufs=4, space="PSUM") as yps2a:
                    for ci, (f0, f1) in enumerate(ch2):
                        fl = f1 - f0
                        ps2a = yps2a.tile([64, FCH], f32, tag="ps")
                        for k in range(2):
                            nc.tensor.matmul(
                                ps2a[:, :fl], W2b_k(k), s2T[:, k, f0:f1],
                                start=(k == 0), stop=(k == 1),
                            )
                        nc.scalar.activation(y2T[:, f0:f1], ps2a[:, :fl], AF.Copy)

                # reduce partials, AllReduce
                stm = sb.tile([128, 2], f32)
                stx = sb.tile([1, 2], f32)
                nc.vector.tensor_reduce(stm[:, 0:1], st1_sum[:], AX.X, OP.add)
                nc.vector.tensor_reduce(stm[:, 1:2], st1_sq[:], AX.X, OP.add)
                nc.vector.tensor_reduce(stx[:, 0:1], st1x_sum[:], AX.X, OP.add)
                nc.vector.tensor_reduce(stx[:, 1:2], st1x_sq[:], AX.X, OP.add)
                ar_in = dram.tile([129, 2], f32, tag="ar1i")
                ar_out = dram.tile([129, 2], f32, tag="ar1o")
                nc.gpsimd.dma_start(ar_in[0:128, :], stm[:])
                nc.gpsimd.dma_start(ar_in[128:129, :], stx[:])
                nc.gpsimd.collective_compute(
                    "AllReduce",
                    OP.add,
                    ins=[ar_in.opt()],
                    outs=[ar_out.opt()],
                    replica_groups=[list(range(NCORES))],
                )
                rst_m = sb.tile([128, 2], f32)
                rst_x = sb.tile([1, 2], f32)
                nc.gpsimd.dma_start(rst_m[:], ar_out[0:128, :])
                nc.gpsimd.dma_start(rst_x[:], ar_out[128:129, :])
                s_m, t_m = bn_scalars(sb, rst_m, bn1, float(N2), 128, "bn1m")
                s_x, t_x = bn_scalars(sb, rst_x, bn1x, float(N2), 1, "bn1x")
                nc.scalar.activation(
                    x1T[:], y1T[:], AF.Lrelu, bias=t_m[:], scale=s_m[:], alpha=SLOPE
                )
                nc.scalar.activation(
                    x1Tx[:], y1Tx[:], AF.Lrelu, bias=t_x[:], scale=s_x[:], alpha=SLOPE
                )

            # ---------------- z2 = x1 @ W2a (point-major) ----------------
            with (
                nc.named_scope("ph_z2"),
                tc.tile_pool(name="z2ps", bufs=4, space="PSUM") as z2ps,
            ):
                for t in range(NZ2R):
                    c0 = t * 128
                    ps = z2ps.tile([128, 64], f32, tag="ps")
                    nc.tensor.matmul(
                        ps[:], x1T[:, c0 : c0 + 128], W2a,
                        start=True, stop=False,
                    )
                    nc.tensor.matmul(
                        ps[:], x1Tx[:, c0 : c0 + 128], W2ax,
                        start=False, stop=True,
                    )
                    nc.scalar.activation(z2pm[:, t, :], ps[:], AF.Copy)

            poolA.close()

            # ---------------- stage 2 (channel-major) ----------------
            with (
                nc.named_scope("ph_st2"),
                tc.tile_pool(name="yps2", bufs=6, space="PSUM") as yps2,
                tc.tile_pool(name="sqscr2", bufs=2) as sqscr2,
            ):
                for ci, (f0, f1) in enumerate(ch2):
                    fl = f1 - f0
                    rr = runs_in(bounds2, f0, f1)
                    if rr:
                        ps = yps2.tile([64, FCH], f32, tag="ps")
                        for mi, (r, a, b) in enumerate(rr):
                            nc.tensor.matmul(
                                ps[:, a - f0 : b - f0], z2pm[:, r, :], P2[:, a:b],
                                start=(mi == 0), stop=(mi == len(rr) - 1),
                                skip_group_check=True,
                            )
                        spans = []
                        for r, a, b in rr:
                            if spans and spans[-1][1] == a:
                                spans[-1][1] = b
                            else:
                                spans.append([a, b])
                        for a, b in spans:
                            nc.vector.tensor_tensor(
                                y2T[:, a:b], y2T[:, a:b],
                                ps[:, a - f0 : b - f0], OP.add,
                            )
                    scr = sqscr2.tile([64, FCH], f16, tag="scr")
                    nc.vector.tensor_reduce(
                        st2_sum[:, ci : ci + 1], y2T[:, f0:f1], AX.X, OP.add
                    )
                    nc.scalar.activation(
                        scr[:, :fl], y2T[:, f0:f1], AF.Square,
                        accum_out=st2_sq[:, ci : ci + 1],
                    )

                st2s = sb.tile([64, 2], f32)
                nc.vector.tensor_reduce(st2s[:, 0:1], st2_sum[:], AX.X, OP.add)
                nc.vector.tensor_reduce(st2s[:, 1:2], st2_sq[:], AX.X, OP.add)
                ar2_in = dram.tile([64, 2], f32, tag="ar2i")
                ar2_out = dram.tile([64, 2], f32, tag="ar2o")
                nc.gpsimd.dma_start(ar2_in[:], st2s[:])
                nc.gpsimd.collective_compute(
                    "AllReduce",
                    OP.add,
                    ins=[ar2_in.opt()],
                    outs=[ar2_out.opt()],
                    replica_groups=[list(range(NCORES))],
                )
                rst2 = sb.tile([64, 2], f32)
                nc.gpsimd.dma_start(rst2[:], ar2_out[:])
                s2s, t2s = bn_scalars(sb, rst2, bn2, float(N1), 64, "bn2")
                nc.scalar.activation(
                    x2T[0:64, :], y2T[:], AF.Lrelu, bias=t2s[:], scale=s2s[:], alpha=SLOPE
                )

            poolB.close()

            # ---------------- stage 3 (paired 2-stack, JIT z3 in psum) -----
            with (
                nc.named_scope("ph_st3"),
                tc.tile_pool(name="s1chunk", bufs=2) as s1chunk,
                tc.tile_pool(name="outp", bufs=4) as outp,
                tc.tile_pool(name="yps3", bufs=6, space="PSUM") as yps3,
            ):
                for (cols0, clen, pieces), ops in zip(s1chunks, chunk_ops):
                    s1c = s1chunk.tile([128, S1CMAX], f16, tag="s1c")
                    nc.sync.dma_start(
                        s1c[:, :clen], s1T_h.ap()[:, cols0 : cols0 + clen]
                    )
                    po0 = ops[0][5]
                    pw = sum(o[2] for o in ops)
                    obuf = outp.tile([98, PWMAX], f16, tag="obuf")
                    for oi, op in enumerate(ops):
                        tag, k, ck, g0, s0, po = op
                        nj = ck // k
                        lp = po - po0
                        a0 = g0 - cols0
                        ps = yps3.tile([98, 512], f32, tag="ps")
                        nc.tensor.matmul(
                            ps[0:34, :ck], W3b, s1c[:, a0 : a0 + ck],
                            start=True, stop=False, skip_group_check=True,
                        )
                        nc.tensor.matmul(
                            ps[0:34, :ck], W3a2a,
                            x2T[:, s0 : s0 + nj]
                            .unsqueeze(2)
                            .broadcast_to([65, nj, k]),
                            start=False, stop=True, skip_group_check=True,
                        )
                        if tag == "pair":
                            nc.tensor.matmul(
                                ps[64:98, :ck], W3b, s1c[:, a0 + ck : a0 + 2 * ck],
                                start=True, stop=False, skip_group_check=True,
                            )
                            nc.tensor.matmul(
                                ps[64:98, :ck], W3a2b,
                                x2T[:, s0 + nj : s0 + 2 * nj]
                                .unsqueeze(2)
                                .broadcast_to([65, nj, k]),
                                start=False, stop=True, skip_group_check=True,
                            )
                            prange = ps[0:98, :ck]
                            orange = obuf[:, lp : lp + ck]
                        else:
                            prange = ps[0:34, :ck]
                            orange = obuf[0:34, lp : lp + ck]
                        h = (ck // 2 + k - 1) // k * k  # split at a point boundary
                        if h < ck:
                            nc.vector.tensor_copy(orange[:, :h], prange[:, :h])
                            nc.scalar.activation(orange[:, h:], prange[:, h:], AF.Copy)
                        elif oi % 2 == 0:
                            nc.vector.tensor_copy(orange, prange)
                        else:
                            nc.scalar.activation(orange, prange, AF.Copy)
                    nc.scalar.dma_start(
                        out_h.ap()[:, po0 : po0 + pw], obuf[:, :pw]
                    )

    nc.compile()
    return nc


# ---------------------------------------------------------------------------
# host wrapper
# ---------------------------------------------------------------------------

_CACHE = {}


def _get_program(key, *args):
    if key not in _CACHE:
        _CACHE[key] = _build_program(*args)
    return _CACHE[key]


def prepare(feats, skip1, skip2, skip3, idx1, idx2, idx3,
            W1, b1, g1, be1, W2, b2, g2, be2, W3, b3):
    feats = np.asarray(feats, np.float32)
    skip1 = np.asarray(skip1, np.float32)
    skip2 = np.asarray(skip2, np.float32)
    skip3 = np.asarray(skip3, np.float32)
    idx1 = np.asarray(idx1, np.int64)
    idx2 = np.asarray(idx2, np.int64)
    idx3 = np.asarray(idx3, np.int64)
    W1 = np.asarray(W1, np.float32)
    W2 = np.asarray(W2, np.float32)
    W3 = np.asarray(W3, np.float32)
    b3 = np.asarray(b3, np.float32)
    g1 = np.asarray(g1, np.float32)
    be1 = np.asarray(be1, np.float32)
    g2 = np.asarray(g2, np.float32)
    be2 = np.asarray(be2, np.float32)

    own1 = idx1 // C3
    P1s, cnt1 = _bucket(own1, NCORES)
    own2 = own1[idx2]
    P2s, cnt2 = _bucket(own2, NCORES)

    # ---- stage-1 slots: rank-major (src_local // 128), padded per rank ----
    rk1 = [None] * NCORES
    for c in range(NCORES):
        sl = idx1[P1s[c]] - C3 * c
        rk1[c] = sl // 128
    R1 = np.zeros(4, np.int64)
    for c in range(NCORES):
        R1 = np.maximum(R1, np.bincount(rk1[c], minlength=4))
    B1 = np.concatenate([[0], np.cumsum(R1)])
    n1p = _ceil_to(int(B1[4]), 128)

    slot1 = np.full(N2, -1, np.int64)  # global stage-1 point -> core slot
    p1_sorted = [None] * NCORES
    for c in range(NCORES):
        order = np.argsort(rk1[c], kind="stable")
        pts = P1s[c][order]
        rks = rk1[c][order]
        slots = np.empty(len(pts), np.int64)
        for r in range(4):
            m = rks == r
            slots[m] = B1[r] + np.arange(int(m.sum()))
        slot1[pts] = slots
        p1_sorted[c] = (pts, slots)

    # ---- stage-3 fanout classes over stage-2 points ----
    fan = np.bincount(idx3, minlength=N1)
    KMAX = int(fan.max())
    NZ2R = n1p // 128

    # cells (class, rank) capacities = max over cores
    cellcnt = np.zeros((NCORES, KMAX + 1, NZ2R), np.int64)
    srcslot2 = np.empty(N1, np.int64)
    for c in range(NCORES):
        pts = P2s[c]
        srcslot2[pts] = slot1[idx2[pts]]
        np.add.at(cellcnt[c], (fan[pts], srcslot2[pts] // 128), 1)
    caps = cellcnt.max(axis=0)  # [K+1, NZ2R]

    # class-major, rank-minor slot layout; M[k] = class block size
    M = caps.sum(axis=1)
    # pad class 0 so n2p is a multiple of FCH
    tot = int(M.sum())
    M[0] += _ceil_to(tot, 2 * FCH) - tot
    Mtuple = tuple(int(x) for x in M)
    src_off, out_off, n2p, n3p, _ = _cls_layout(list(Mtuple), KMAX)

    # cell start offsets within class blocks (pad of class 0 at block end)
    cell_start = np.zeros((KMAX + 1, NZ2R), np.int64)
    cells2 = []
    for k in range(KMAX + 1):
        o = src_off[k]
        for r in range(NZ2R):
            cell_start[k, r] = o
            if caps[k, r] > 0:
                cells2.append((int(r), int(o), int(caps[k, r])))
            o += caps[k, r]

    # per-core stage-2 slot assignment
    slot2 = np.full(N1, -1, np.int64)
    for c in range(NCORES):
        pts = P2s[c]
        key = fan[pts] * NZ2R + srcslot2[pts] // 128
        order = np.argsort(key, kind="stable")
        pts = pts[order]
        keys = key[order]
        slots = np.empty(len(pts), np.int64)
        uk, inv, ucnt = np.unique(keys, return_inverse=True, return_counts=True)
        for ui, kk in enumerate(uk):
            m = inv == ui
            slots[m] = cell_start[kk // NZ2R, kk % NZ2R] + np.arange(int(m.sum()))
        slot2[pts] = slots

    # stage-3 output mapping (CSR over idx3 by source)
    order3 = np.argsort(idx3, kind="stable")
    start = np.zeros(N1 + 1, np.int64)
    np.cumsum(fan, out=start[1:])

    outmaps = []
    for c in range(NCORES):
        omap = np.full(n3p, -1, np.int64)
        pts = P2s[c]
        ks = fan[pts]
        for k in range(1, KMAX + 1):
            m = ks == k
            if not m.any():
                continue
            srcs = pts[m]
            j = slot2[srcs] - src_off[k]  # position within class block
            gidx = (start[srcs][:, None] + np.arange(k)[None, :])
            cols = (out_off[k] + j[:, None] * k + np.arange(k)[None, :])
            omap[cols.reshape(-1)] = order3[gidx.reshape(-1)]
        outmaps.append(omap)

    key = (n1p, tuple(int(x) for x in B1), n2p, tuple(cells2), Mtuple, KMAX)

    # ---- packed param blobs ----
    blob0 = np.zeros((128, 2929), np.float16)
    blob0[:, 0:129] = W1[0:128]
    blob0[:, 129:258] = W1[128:256]
    blob0[0:2, 258:387] = W1[256:258]
    for k in range(4):
        blob0[:, 387 + k * 129 : 387 + (k + 1) * 129] = W1[258 + 128 * k : 258 + 128 * (k + 1)]
    blob0[:, 903:967] = W2[0:128]
    blob0[0:1, 967:1031] = W2[128:129]
    for k in range(2):
        blob0[:, 1031 + k * 64 : 1031 + (k + 1) * 64] = W2[129 + 128 * k : 129 + 128 * (k + 1)]
    blob0[0:64, 1159:1193] = W3[0:64]
    blob0[:, 1193:1227] = W3[64:192]
    blob0[0:64, 2763:2797] = W3[0:64]
    blob0[64:65, 2763:2797] = b3.reshape(1, 34)
    blob0[0:64, 2797 + 32 : 2863] = W3[0:64]
    blob0[64:65, 2797 + 32 : 2863] = b3.reshape(1, 34)
    blobf = np.zeros((128, 200), np.float32)
    blobf[:, 0:2] = np.stack([g1[0:128], be1[0:128]], 1)
    blobf[0:1, 2:4] = np.stack([g1[128:129], be1[128:129]], 1)
    blobf[0:64, 4:6] = np.stack([g2, be2], 1)
    blobf[0:34, 7:8] = b3.reshape(34, 1)
    blobf[64:98, 7:8] = b3.reshape(34, 1)
    eye = np.eye(64, dtype=np.float32)
    blobf[0:64, 8:72] = eye
    blobf[64:128, 8:72] = eye
    blobf[0:64, 72:136] = eye
    blobf[0:64, 136:200] = eye

    featsTf = np.ascontiguousarray(feats.T)
    s3Tf = skip3.T
    s2Tf = skip2.T
    s1Tf = skip1.T

    in_maps = []
    for c in range(NCORES):
        pts1, slots1 = p1_sorted[c]
        s3T = np.zeros((512, n1p), np.float16)
        s3T[:, slots1] = s3Tf[:, pts1]
        P1m = np.zeros((128, n1p), np.float16)
        P1m[(idx1[pts1] - C3 * c) % 128, slots1] = 1

        pts2 = P2s[c]
        sl2 = slot2[pts2]
        s2T = np.zeros((257, n2p), np.float16)
        s2T[0:256, sl2] = s2Tf[:, pts2]
        s2T[256, :] = 1
        P2m = np.zeros((128, n2p), np.float16)
        P2m[srcslot2[pts2] % 128, sl2] = 1

        omap = outmaps[c]
        valid = omap >= 0
        s1T = np.zeros((128, n3p), np.float16)
        s1T[:, valid] = s1Tf[:, omap[valid]]

        blob = blob0.copy()
        fc = featsTf[:, C3 * c : C3 * (c + 1)].astype(np.float16)
        blob[:, 1227:1739] = fc[0:128]
        blob[:, 1739:2251] = fc[128:256]
        blob[0:2, 2251:2763] = fc[256:258]
        in_maps.append(
            {
                "s3T": s3T,
                "s2T": s2T,
                "s1T": s1T,
                "P1": P1m,
                "P2": P2m,
                "blob": blob,
                "blobf": blobf,
            }
        )

    return key, in_maps, outmaps


def _install_ntff_hook():
    import sys as _sys
    import types

    if "antenv.axon_hooks" in _sys.modules:
        return
    mod = types.ModuleType("antenv.axon_hooks")
    holder = {}
    mod.set_axon_ntff_profile_hook = lambda h: holder.__setitem__("h", h)
    mod.get_axon_ntff_profile_hook = lambda: holder.get("h")
    _sys.modules["antenv.axon_hooks"] = mod
    try:
        from trn_agent_boot.trn_boot import _ntff_profile_via_ctypes

        h = _ntff_profile_via_ctypes("/opt/axon/libaxon_pjrt.so")
        if h is not None:
            holder["h"] = h
    except Exception:
        pass


def kernel(_want_trace=False, **inputs):
    if _want_trace:
        _install_ntff_hook()
    key, in_maps, outmaps = prepare(**inputs)
    nc = _get_program(key, *key)

    res = bass_utils.run_bass_kernel_spmd(
        nc, in_maps, core_ids=list(range(NCORES)), trace=_want_trace
    )

    n1p, B1, n2p, cells2, Mtuple, KMAX = key
    _, _, _, n3p, s1chunks = _cls_layout(list(Mtuple), KMAX)
    chunk_ops, PK = _pair_ops(s1chunks)
    out = np.empty((N0, 34), np.float32)
    for c in range(NCORES):
        r = res.results[c]["out"]  # [98, PK]
        o34 = np.empty((34, n3p), np.float32)
        for ops in chunk_ops:
            for tag, k, ck, g0, s0, po in ops:
                o34[:, g0 : g0 + ck] = r[0:34, po : po + ck]
                if tag == "pair":
                    o34[:, g0 + ck : g0 + 2 * ck] = r[64:98, po : po + ck]
        omap = outmaps[c]
        valid = omap >= 0
        out[omap[valid]] = o34.T[valid]

    if _want_trace:
        kernel._last_trace = res
    return out


# revision 3
# speedup vs baseline: 1.0352x; 1.0352x over previous
"""Trainium2 Bass kernel for nn_Decoder — v2, gather-free.

Architecture (8-core SPMD, one shared program):
  - Gather commutes with the per-row linear map: stages pre-multiply the
    gathered operand into small z-tables (z1 = feats@W1a, z2 = x1@W2a,
    z3 = x2@W3a), kept point-major in SBUF.
  - Upsample expansions are ONE-HOT MATMULS on the tensor engine:
    channel-major psum[ch, cols] += z_pm_rank.T @ P[:, cols], where P is a
    host-built fp16 one-hot matrix (row = source slot % 128) and points are
    sorted so each run of columns draws from a single 128-slot source rank.
    No DMA gathers anywhere.
  - Stage-2 points are laid out class-major (stage-3 fanout class), rank-minor,
    padded per (class, rank) cell to the max across cores so the program is
    shared. Stage-3 expansion is the stride-0 class-broadcast DVE/GPSIMD add.
  - Everything HBM-facing is fp16 (host casts): skips, weights, one-hots,
    output. Matmuls run fp16 (4x over fp32); psum/BN stats stay fp32.
  - BN stats per chunk from PSUM; two tiny AllReduces; BN-affine + LeakyReLU
    fused in one ACT op with per-partition scale/bias.
"""

import sys

sys.path.insert(0, "/opt/trn_rl_repo")

import numpy as np

from concourse import bacc, bass_utils, mybir, tile

dt = mybir.dt
AF = mybir.ActivationFunctionType
OP = mybir.AluOpType
AX = mybir.AxisListType

NCORES = 8
EPS = 1e-5
SLOPE = 0.01

N3, N2, N1, N0 = 4096, 16384, 65536, 262144
C3 = 512  # feats rows per core

S1CMAX = 10240  # stage-3 skip chunk columns resident in SBUF
FCH = 512  # psum chunk width


def _ceil_to(x, m):
    return ((x + m - 1) // m) * m


def _bucket(owner, ncores):
    order = np.argsort(owner, kind="stable")
    counts = np.bincount(owner, minlength=ncores)
    splits = np.split(order, np.cumsum(counts)[:-1])
    return splits, counts


def _cls_layout(M, kmax):
    """M[k] = class-k slot capacity. Returns (src_off, out_off, n2p, n3p,
    s1chunks) where s1chunks groups class pieces into <=S1CMAX output-column
    loads, pieces of <=512 cols each."""
    K = len(M)
    src_off = [0] * K
    o = 0
    for k in range(K):
        src_off[k] = o
        o += M[k]
    n2p = o
    out_off = [0] * K
    o = 0
    for k in range(1, K):
        out_off[k] = o
        o += M[k] * k
    n3p = o

    chunks = []
    cur0, curlen, curpieces = 0, 0, []
    for k in range(1, K):
        blk = M[k] * k
        if blk == 0:
            continue
        if curlen + blk > S1CMAX and curlen > 0:
            chunks.append((cur0, curlen, curpieces))
            cur0, curlen, curpieces = out_off[k], 0, []
        ck_max = (512 // k) * k
        done = 0
        while done < blk:
            ck = min(ck_max, blk - done)
            curpieces.append((k, out_off[k] + done, src_off[k] + done // k, ck))
            done += ck
        curlen += blk
    if curlen > 0:
        chunks.append((cur0, curlen, curpieces))
    return src_off, out_off, n2p, n3p, chunks


def _pair_ops(s1chunks):
    """Pair consecutive same-(k, ck) pieces within each chunk for 2-stack
    processing. Returns (chunk_ops, PK): chunk_ops[i] = list of
    ("pair", k, ck, gA, s0A, po) | ("single", k, ck, g0, s0, po);
    PK = packed out_h column count."""
    chunk_ops = []
    po = 0
    for cols0, clen, pieces in s1chunks:
        ops = []
        i = 0
        while i < len(pieces):
            k, g0, s0, ck = pieces[i]
            if i + 1 < len(pieces):
                k2, g2, s2, ck2 = pieces[i + 1]
                if k2 == k and ck2 == ck and g2 == g0 + ck:
                    ops.append(("pair", k, ck, g0, s0, po))  # gB=g0+ck, s0B=s0+nj
                    po += ck
                    i += 2
                    continue
            ops.append(("single", k, ck, g0, s0, po))
            po += ck
            i += 1
        chunk_ops.append(ops)
    return chunk_ops, po


# ---------------------------------------------------------------------------
# device program
# ---------------------------------------------------------------------------


def _build_program(n1p, B1, n2p, cells2, Mtuple, kmax):
    """B1: tuple of 5 cumulative stage-1 rank boundaries (incl end of real
    slots). cells2: tuple of (rank, start, cap) runs over stage-2 slots,
    class-major order. Mtuple: per-class slot capacities for stage 3."""
    src_off, out_off, n2p_c, n3p, s1chunks = _cls_layout(list(Mtuple), kmax)
    assert n2p_c <= n2p

    nc = bacc.Bacc(
        "TRN2",
        target_bir_lowering=False,
        debug=False,
        num_devices=NCORES,
        num_swdge_queues=4,
    )

    f16 = dt.float16
    f32 = dt.float32

    NZ1R = 4  # feats source ranks
    NZ2R = n1p // 128  # stage-1 slot ranks
    BLOB16 = 2929  # packed fp16 params+feats (see host packing)
    chunk_ops, PK = _pair_ops(s1chunks)
    PWMAX = max(sum(o[2] for o in ops) for ops in chunk_ops)

    # ---- I/O (fp16 unless noted) ----
    s3T_h = nc.dram_tensor("s3T", [512, n1p], f16, kind="ExternalInput")
    s2T_h = nc.dram_tensor("s2T", [257, n2p], f16, kind="ExternalInput")
    s1T_h = nc.dram_tensor("s1T", [128, n3p], f16, kind="ExternalInput")
    P1_h = nc.dram_tensor("P1", [128, n1p], f16, kind="ExternalInput")
    P2_h = nc.dram_tensor("P2", [128, n2p], f16, kind="ExternalInput")
    blob_h = nc.dram_tensor("blob", [128, BLOB16], f16, kind="ExternalInput")
    blobf_h = nc.dram_tensor("blobf", [128, 200], f32, kind="ExternalInput")
    out_h = nc.dram_tensor("out", [98, PK], f16, kind="ExternalOutput")

    def bn_scalars(sb, stats, gbe, n_true, P, name):
        """stats [P,2]=(sum,sumsq) -> (s, t): s=g*rsqrt(var+eps), t=be-mean*s."""
        mean = sb.tile([P, 1], f32, tag=f"{name}_mean")
        ms = sb.tile([P, 1], f32, tag=f"{name}_ms")
        nc.vector.tensor_scalar(mean[:], stats[:, 0:1], 1.0 / n_true, None, OP.mult)
        nc.vector.tensor_scalar(ms[:], stats[:, 1:2], 1.0 / n_true, None, OP.mult)
        var = sb.tile([P, 1], f32, tag=f"{name}_var")
        nc.vector.tensor_tensor(var[:], mean[:], mean[:], OP.mult)
        nc.vector.tensor_tensor(var[:], ms[:], var[:], OP.subtract)
        nc.vector.tensor_scalar(var[:], var[:], EPS, None, OP.add)
        std = sb.tile([P, 1], f32, tag=f"{name}_std")
        nc.scalar.activation(std[:], var[:], AF.Sqrt)
        s = sb.tile([P, 1], f32, tag=f"{name}_s")
        nc.vector.reciprocal(s[:], std[:])
        nc.vector.tensor_tensor(s[:], s[:], gbe[:, 0:1], OP.mult)
        t = sb.tile([P, 1], f32, tag=f"{name}_t")
        nc.vector.tensor_tensor(t[:], mean[:], s[:], OP.mult)
        nc.vector.tensor_tensor(t[:], gbe[:, 1:2], t[:], OP.subtract)
        return s, t

    def chunks_of(n, step=FCH):
        out = []
        o = 0
        while o < n:
            out.append((o, min(o + step, n)))
            o += step
        return out

    def runs_in(bounds, f0, f1):
        """bounds: list of (rank, a, b) slot ranges. Clip to [f0, f1)."""
        out = []
        for r, a, b in bounds:
            a2, b2 = max(a, f0), min(b, f1)
            if a2 < b2:
                out.append((r, a2, b2))
        return out

    bounds1 = [(r, B1[r], B1[r + 1]) for r in range(NZ1R)]
    bounds2 = [(r, a, a + cap) for (r, a, cap) in cells2]

    with tile.TileContext(nc) as tc:
        from contextlib import ExitStack

        octx = ExitStack()
        with octx:
            sb = octx.enter_context(tc.tile_pool(name="persist", bufs=1))
            dram = octx.enter_context(tc.tile_pool(name="dram", bufs=1, space="DRAM"))

            # ---- packed params (one DMA each) ----
            blob = sb.tile([128, BLOB16], f16)
            nc.sync.dma_start(blob[:], blob_h.ap())
            blobf = sb.tile([128, 200], f32)
            nc.sync.dma_start(blobf[:], blobf_h.ap())
            Mhalf = blobf[:, 8:72]      # [128, 64]: out[c] = in[c] + in[c+64]
            MTrep = blobf[0:64, 72:200]  # [64, 128]: replicate [64] -> [128]
            b3stk = blobf[:, 7:8]        # b3 at rows 0-33 and 64-97

            def W1a_k(k):
                return blob[:, k * 129 : (k + 1) * 129]

            W1ax = blob[0:2, 258:387]

            def W1b_k(k):
                return blob[:, 387 + k * 129 : 387 + k * 129 + 128]

            def W1bx_k(k):
                return blob[:, 387 + k * 129 + 128 : 387 + (k + 1) * 129]

            W2a = blob[:, 903:967]
            W2ax = blob[0:1, 967:1031]

            def W2b_k(k):
                return blob[:, 1031 + k * 64 : 1031 + (k + 1) * 64]

            W3a = blob[0:64, 1159:1193]
            W3b = blob[:, 1193:1227]

            def featsT_k(k):
                return blob[:, 1227 + k * 512 : 1227 + (k + 1) * 512]

            featsTx = blob[0:2, 2251:2763]
            W3a2a = blob[0:65, 2763:2797]
            W3a2b = blob[0:65, 2829:2863]
            bn1 = blobf[:, 0:2]
            bn1x = blobf[0:1, 2:4]
            bn2 = blobf[0:64, 4:6]
            b3p = blobf[0:34, 6:7]

            # ---- pools: poolB (stage-2 inputs) outer, poolA (stage-1) inner
            poolB = ExitStack()  # dies after st2
            pB = poolB.enter_context(tc.tile_pool(name="poolB", bufs=1))
            P2 = pB.tile([128, n2p], f16)
            s2T = pB.tile([128, 2, n2p], f16)
            y2T = pB.tile([64, n2p], f16)
            poolA = ExitStack()  # dies after z2
            pA = poolA.enter_context(tc.tile_pool(name="poolA", bufs=1))
            P1 = pA.tile([128, n1p], f16)
            s3T = pA.tile([128, 4, n1p], f16)

            # loads in priority order (stage-1 inputs first)
            nc.sync.dma_start(P1[:], P1_h.ap())
            for k in range(4):
                nc.sync.dma_start(s3T[:, k, :], s3T_h.ap()[k * 128 : (k + 1) * 128, :])
            nc.sync.dma_start(P2[:], P2_h.ap())
            for k in range(2):
                nc.sync.dma_start(s2T[:, k, :], s2T_h.ap()[k * 128 : (k + 1) * 128, :])

            # pre-warm ACT function tables off the critical path
            twarm = sb.tile([1, 4], f32)
            nc.vector.memzero(twarm[:])
            nc.scalar.activation(twarm[:, 0:1], twarm[:, 1:2], AF.Square)
            nc.scalar.activation(twarm[:, 0:1], twarm[:, 1:2], AF.Sqrt)
            nc.scalar.activation(twarm[:, 0:1], twarm[:, 1:2], AF.Lrelu, alpha=SLOPE)

            # ---- z tables + cross-stage tiles ----
            z1pm = sb.tile([128, NZ1R, 129], f16)
            z2pm = sb.tile([128, NZ2R, 64], f16)
            x2T = sb.tile([65, n2p], f16)
            nc.sync.dma_start(x2T[64:65, :], s2T_h.ap()[256:257, :])
            y1T = pA.tile([128, n1p], f16)
            y1Tx = pA.tile([1, n1p], f16)
            x1T = pA.tile([128, n1p], f16)
            x1Tx = pA.tile([1, n1p], f16)

            ch1 = chunks_of(n1p)
            ch2 = chunks_of(n2p)
            st1_sum = sb.tile([128, len(ch1)], f32)
            st1_sq = sb.tile([128, len(ch1)], f32)
            st1x_sum = sb.tile([1, len(ch1)], f32)
            st1x_sq = sb.tile([1, len(ch1)], f32)
            st2_sum = sb.tile([64, len(ch2)], f32)
            st2_sq = sb.tile([64, len(ch2)], f32)

            # ---------------- z1 = feats @ W1a (point-major) ----------------
            with (
                nc.named_scope("ph_z1"),
                tc.tile_pool(name="z1ps", bufs=2, space="PSUM") as z1ps,
            ):
                for t in range(NZ1R):
                    ps = z1ps.tile([128, 129], f32, tag="ps")
                    c0 = t * 128
                    nc.tensor.matmul(
                        ps[:], featsT_k(0)[:, c0 : c0 + 128], W1a_k(0),
                        start=True, stop=False,
                    )
                    nc.tensor.matmul(
                        ps[:], featsT_k(1)[:, c0 : c0 + 128], W1a_k(1),
                        start=False, stop=False,
                    )
                    nc.tensor.matmul(
                        ps[:], featsTx[:, c0 : c0 + 128], W1ax,
                        start=False, stop=True,
                    )
                    nc.scalar.activation(z1pm[:, t, :], ps[:], AF.Copy)

            # ---------------- stage 1 (channel-major) ----------------
            with (
                nc.named_scope("ph_st1"),
                tc.tile_pool(name="yps", bufs=2, space="PSUM") as yps,
                tc.tile_pool(name="ypsx", bufs=2, space="PSUM") as ypsx,
                tc.tile_pool(name="sqscr", bufs=2) as sqscr,
            ):
                for ci, (f0, f1) in enumerate(ch1):
                    fl = f1 - f0
                    ps = yps.tile([128, FCH], f32, tag="ps")
                    psx = ypsx.tile([1, FCH], f32, tag="psx")
                    rr = runs_in(bounds1, f0, f1)
                    mms = [(ps, W1b_k(k), s3T[:, k, f0:f1], 0, fl) for k in range(4)]
                    mms += [(ps, z1pm[:, r, 0:128], P1[:, a:b], a - f0, b - f0) for r, a, b in rr]
                    mmx = [(psx, W1bx_k(k), s3T[:, k, f0:f1], 0, fl) for k in range(4)]
                    mmx += [(psx, z1pm[:, r, 128:129], P1[:, a:b], a - f0, b - f0) for r, a, b in rr]
                    for group in (mms, mmx):
                        for mi, (po, lhs, rhs, a, b) in enumerate(group):
                            nc.tensor.matmul(
                                po[:, a:b], lhs, rhs,
                                start=(mi == 0), stop=(mi == len(group) - 1),
                                skip_group_check=True,
                            )
                    # stats + copy out of psum
                    scr = sqscr.tile([128, FCH], f16, tag="scr")
                    scrx = sqscr.tile([1, FCH], f16, tag="scrx")
                    nc.vector.tensor_reduce(
                        st1_sum[:, ci : ci + 1], ps[:, :fl], AX.X, OP.add
                    )
                    nc.scalar.activation(
                        scr[:, :fl], ps[:, :fl], AF.Square,
                        accum_out=st1_sq[:, ci : ci + 1],
                    )
                    nc.scalar.activation(y1T[:, f0:f1], ps[:, :fl], AF.Copy)
                    nc.vector.tensor_reduce(
                        st1x_sum[:, ci : ci + 1], psx[:, :fl], AX.X, OP.add
                    )
                    nc.scalar.activation(
                        scrx[:, :fl], psx[:, :fl], AF.Square,
                        accum_out=st1x_sq[:, ci : ci + 1],
                    )
                    nc.scalar.activation(y1Tx[:, f0:f1], psx[:, :fl], AF.Copy)

                # ---- stage-2 partial (W2b x s2T): fills the AR1 window ----
                with tc.tile_pool(name="yps2a", bufs=4, space="PSUM") as yps2a:
                    for ci, (f0, f1) in enumerate(ch2):
                        fl = f1 - f0
                        ps2a = yps2a.tile([64, FCH], f32, tag="ps")
                        for k in range(2):
                            nc.tensor.matmul(
                                ps2a[:, :fl], W2b_k(k), s2T[:, k, f0:f1],
                                start=(k == 0), stop=(k == 1),
                            )
                        nc.scalar.activation(y2T[:, f0:f1], ps2a[:, :fl], AF.Copy)

                # reduce partials, AllReduce
                stm = sb.tile([128, 2], f32)
                stx = sb.tile([1, 2], f32)
                nc.vector.tensor_reduce(stm[:, 0:1], st1_sum[:], AX.X, OP.add)
                nc.vector.tensor_reduce(stm[:, 1:2], st1_sq[:], AX.X, OP.add)
                nc.vector.tensor_reduce(stx[:, 0:1], st1x_sum[:], AX.X, OP.add)
                nc.vector.tensor_reduce(stx[:, 1:2], st1x_sq[:], AX.X, OP.add)
                ar_in = dram.tile([129, 2], f32, tag="ar1i")
                ar_out = dram.tile([129, 2], f32, tag="ar1o")
                nc.gpsimd.dma_start(ar_in[0:128, :], stm[:])
                nc.gpsimd.dma_start(ar_in[128:129, :], stx[:])
                nc.gpsimd.collective_compute(
                    "AllReduce",
                    OP.add,
                    ins=[ar_in.opt()],
                    outs=[ar_out.opt()],
                    replica_groups=[list(range(NCORES))],
                )
                rst_m = sb.tile([128, 2], f32)
                rst_x = sb.tile([1, 2], f32)
                nc.gpsimd.dma_start(rst_m[:], ar_out[0:128, :])
                nc.gpsimd.dma_start(rst_x[:], ar_out[128:129, :])
                s_m, t_m = bn_scalars(sb, rst_m, bn1, float(N2), 128, "bn1m")
                s_x, t_x = bn_scalars(sb, rst_x, bn1x, float(N2), 1, "bn1x")
                nc.scalar.activation(
                    x1T[:], y1T[:], AF.Lrelu, bias=t_m[:], scale=s_m[:], alpha=SLOPE
                )
                nc.scalar.activation(
                    x1Tx[:], y1Tx[:], AF.Lrelu, bias=t_x[:], scale=s_x[:], alpha=SLOPE
                )

            # ---------------- z2 = x1 @ W2a (point-major) ----------------
            with (
                nc.named_scope("ph_z2"),
                tc.tile_pool(name="z2ps", bufs=4, space="PSUM") as z2ps,
            ):
                for t in range(NZ2R):
                    c0 = t * 128
                    ps = z2ps.tile([128, 64], f32, tag="ps")
                    nc.tensor.matmul(
                        ps[:], x1T[:, c0 : c0 + 128], W2a,
                        start=True, stop=False,
                    )
                    nc.tensor.matmul(
                        ps[:], x1Tx[:, c0 : c0 + 128], W2ax,
                        start=False, stop=True,
                    )
                    nc.scalar.activation(z2pm[:, t, :], ps[:], AF.Copy)

            poolA.close()

            # ---------------- stage 2 (channel-major) ----------------
            with (
                nc.named_scope("ph_st2"),
                tc.tile_pool(name="yps2", bufs=6, space="PSUM") as yps2,
                tc.tile_pool(name="sqscr2", bufs=2) as sqscr2,
            ):
                for ci, (f0, f1) in enumerate(ch2):
                    fl = f1 - f0
                    rr = runs_in(bounds2, f0, f1)
                    if rr:
                        ps = yps2.tile([64, FCH], f32, tag="ps")
                        for mi, (r, a, b) in enumerate(rr):
                            nc.tensor.matmul(
                                ps[:, a - f0 : b - f0], z2pm[:, r, :], P2[:, a:b],
                                start=(mi == 0), stop=(mi == len(rr) - 1),
                                skip_group_check=True,
                            )
                        spans = []
                        for r, a, b in rr:
                            if spans and spans[-1][1] == a:
                                spans[-1][1] = b
                            else:
                                spans.append([a, b])
                        for a, b in spans:
                            nc.vector.tensor_tensor(
                                y2T[:, a:b], y2T[:, a:b],
                                ps[:, a - f0 : b - f0], OP.add,
                            )
                    scr = sqscr2.tile([64, FCH], f16, tag="scr")
                    nc.vector.tensor_reduce(
                        st2_sum[:, ci : ci + 1], y2T[:, f0:f1], AX.X, OP.add
                    )
                    nc.scalar.activation(
                        scr[:, :fl], y2T[:, f0:f1], AF.Square,
                        accum_out=st2_sq[:, ci : ci + 1],
                    )

                st2s = sb.tile([64, 2], f32)
                nc.vector.tensor_reduce(st2s[:, 0:1], st2_sum[:], AX.X, OP.add)
                nc.vector.tensor_reduce(st2s[:, 1:2], st2_sq[:], AX.X, OP.add)
                ar2_in = dram.tile([64, 2], f32, tag="ar2i")
                ar2_out = dram.tile([64, 2], f32, tag="ar2o")
                nc.gpsimd.dma_start(ar2_in[:], st2s[:])
                nc.gpsimd.collective_compute(
                    "AllReduce",
                    OP.add,
                    ins=[ar2_in.opt()],
                    outs=[ar2_out.opt()],
                    replica_groups=[list(range(NCORES))],
                )
                rst2 = sb.tile([64, 2], f32)
                nc.gpsimd.dma_start(rst2[:], ar2_out[:])
                s2s, t2s = bn_scalars(sb, rst2, bn2, float(N1), 64, "bn2")
                nq = n2p // 4
                for q in range(4):
                    nc.scalar.activation(
                        x2T[0:64, q * nq : (q + 1) * nq],
                        y2T[:, q * nq : (q + 1) * nq],
                        AF.Lrelu, bias=t2s[:], scale=s2s[:], alpha=SLOPE,
                    )

            poolB.close()

            # ---------------- stage 3 (paired 2-stack, JIT z3 in psum) -----
            with (
                nc.named_scope("ph_st3"),
                tc.tile_pool(name="s1chunk", bufs=2) as s1chunk,
                tc.tile_pool(name="outp", bufs=4) as outp,
                tc.tile_pool(name="yps3", bufs=6, space="PSUM") as yps3,
            ):
                for (cols0, clen, pieces), ops in zip(s1chunks, chunk_ops):
                    s1c = s1chunk.tile([128, S1CMAX], f16, tag="s1c")
                    nc.sync.dma_start(
                        s1c[:, :clen], s1T_h.ap()[:, cols0 : cols0 + clen]
                    )
                    po0 = ops[0][5]
                    pw = sum(o[2] for o in ops)
                    obuf = outp.tile([98, PWMAX], f16, tag="obuf")
                    for oi, op in enumerate(ops):
                        tag, k, ck, g0, s0, po = op
                        nj = ck // k
                        lp = po - po0
                        a0 = g0 - cols0
                        ps = yps3.tile([98, 512], f32, tag="ps")
                        nc.tensor.matmul(
                            ps[0:34, :ck], W3b, s1c[:, a0 : a0 + ck],
                            start=True, stop=False, skip_group_check=True,
                        )
                        nc.tensor.matmul(
                            ps[0:34, :ck], W3a2a,
                            x2T[:, s0 : s0 + nj]
                            .unsqueeze(2)
                            .broadcast_to([65, nj, k]),
                            start=False, stop=True, skip_group_check=True,
                        )
                        if tag == "pair":
                            nc.tensor.matmul(
                                ps[64:98, :ck], W3b, s1c[:, a0 + ck : a0 + 2 * ck],
                                start=True, stop=False, skip_group_check=True,
                            )
                            nc.tensor.matmul(
                                ps[64:98, :ck], W3a2b,
                                x2T[:, s0 + nj : s0 + 2 * nj]
                                .unsqueeze(2)
                                .broadcast_to([65, nj, k]),
                                start=False, stop=True, skip_group_check=True,
                            )
                            prange = ps[0:98, :ck]
                            orange = obuf[:, lp : lp + ck]
                        else:
                            prange = ps[0:34, :ck]
                            orange = obuf[0:34, lp : lp + ck]
                        h = (ck // 2 + k - 1) // k * k  # split at a point boundary
                        if h < ck:
                            nc.vector.tensor_copy(orange[:, :h], prange[:, :h])
                            nc.scalar.activation(orange[:, h:], prange[:, h:], AF.Copy)
                        elif oi % 2 == 0:
                            nc.vector.tensor_copy(orange, prange)
                        else:
                            nc.scalar.activation(orange, prange, AF.Copy)
                    nc.scalar.dma_start(
                        out_h.ap()[:, po0 : po0 + pw], obuf[:, :pw]
                    )

    nc.compile()
    return nc


# ---------------------------------------------------------------------------
# host wrapper
# ---------------------------------------------------------------------------

_CACHE = {}


def _get_program(key, *args):
    if key not in _CACHE:
        _CACHE[key] = _build_program(*args)
    return _CACHE[key]


def prepare(feats, skip1, skip2, skip3, idx1, idx2, idx3,
            W1, b1, g1, be1, W2, b2, g2, be2, W3, b3):
    feats = np.asarray(feats, np.float32)
    skip1 = np.asarray(skip1, np.float32)
    skip2 = np.asarray(skip2, np.float32)
    skip3 = np.asarray(skip3, np.float32)
    idx1 = np.asarray(idx1, np.int64)
    idx2 = np.asarray(idx2, np.int64)
    idx3 = np.asarray(idx3, np.int64)
    W1 = np.asarray(W1, np.float32)
    W2 = np.asarray(W2, np.float32)
    W3 = np.asarray(W3, np.float32)
    b3 = np.asarray(b3, np.float32)
    g1 = np.asarray(g1, np.float32)
    be1 = np.asarray(be1, np.float32)
    g2 = np.asarray(g2, np.float32)
    be2 = np.asarray(be2, np.float32)

    own1 = idx1 // C3
    P1s, cnt1 = _bucket(own1, NCORES)
    own2 = own1[idx2]
    P2s, cnt2 = _bucket(own2, NCORES)

    # ---- stage-1 slots: rank-major (src_local // 128), padded per rank ----
    rk1 = [None] * NCORES
    for c in range(NCORES):
        sl = idx1[P1s[c]] - C3 * c
        rk1[c] = sl // 128
    R1 = np.zeros(4, np.int64)
    for c in range(NCORES):
        R1 = np.maximum(R1, np.bincount(rk1[c], minlength=4))
    B1 = np.concatenate([[0], np.cumsum(R1)])
    n1p = _ceil_to(int(B1[4]), 128)

    slot1 = np.full(N2, -1, np.int64)  # global stage-1 point -> core slot
    p1_sorted = [None] * NCORES
    for c in range(NCORES):
        order = np.argsort(rk1[c], kind="stable")
        pts = P1s[c][order]
        rks = rk1[c][order]
        slots = np.empty(len(pts), np.int64)
        for r in range(4):
            m = rks == r
            slots[m] = B1[r] + np.arange(int(m.sum()))
        slot1[pts] = slots
        p1_sorted[c] = (pts, slots)

    # ---- stage-3 fanout classes over stage-2 points ----
    fan = np.bincount(idx3, minlength=N1)
    KMAX = int(fan.max())
    NZ2R = n1p // 128

    # cells (class, rank) capacities = max over cores
    cellcnt = np.zeros((NCORES, KMAX + 1, NZ2R), np.int64)
    srcslot2 = np.empty(N1, np.int64)
    for c in range(NCORES):
        pts = P2s[c]
        srcslot2[pts] = slot1[idx2[pts]]
        np.add.at(cellcnt[c], (fan[pts], srcslot2[pts] // 128), 1)
    caps = cellcnt.max(axis=0)  # [K+1, NZ2R]

    # class-major, rank-minor slot layout; M[k] = class block size
    M = caps.sum(axis=1)
    # pad class 0 so n2p is a multiple of FCH
    tot = int(M.sum())
    M[0] += _ceil_to(tot, 2 * FCH) - tot
    Mtuple = tuple(int(x) for x in M)
    src_off, out_off, n2p, n3p, _ = _cls_layout(list(Mtuple), KMAX)

    # cell start offsets within class blocks (pad of class 0 at block end)
    cell_start = np.zeros((KMAX + 1, NZ2R), np.int64)
    cells2 = []
    for k in range(KMAX + 1):
        o = src_off[k]
        for r in range(NZ2R):
            cell_start[k, r] = o
            if caps[k, r] > 0:
                cells2.append((int(r), int(o), int(caps[k, r])))
            o += caps[k, r]

    # per-core stage-2 slot assignment
    slot2 = np.full(N1, -1, np.int64)
    for c in range(NCORES):
        pts = P2s[c]
        key = fan[pts] * NZ2R + srcslot2[pts] // 128
        order = np.argsort(key, kind="stable")
        pts = pts[order]
        keys = key[order]
        slots = np.empty(len(pts), np.int64)
        uk, inv, ucnt = np.unique(keys, return_inverse=True, return_counts=True)
        for ui, kk in enumerate(uk):
            m = inv == ui
            slots[m] = cell_start[kk // NZ2R, kk % NZ2R] + np.arange(int(m.sum()))
        slot2[pts] = slots

    # stage-3 output mapping (CSR over idx3 by source)
    order3 = np.argsort(idx3, kind="stable")
    start = np.zeros(N1 + 1, np.int64)
    np.cumsum(fan, out=start[1:])

    outmaps = []
    for c in range(NCORES):
        omap = np.full(n3p, -1, np.int64)
        pts = P2s[c]
        ks = fan[pts]
        for k in range(1, KMAX + 1):
            m = ks == k
            if not m.any():
                continue
            srcs = pts[m]
            j = slot2[srcs] - src_off[k]  # position within class block
            gidx = (start[srcs][:, None] + np.arange(k)[None, :])
            cols = (out_off[k] + j[:, None] * k + np.arange(k)[None, :])
            omap[cols.reshape(-1)] = order3[gidx.reshape(-1)]
        outmaps.append(omap)

    key = (n1p, tuple(int(x) for x in B1), n2p, tuple(cells2), Mtuple, KMAX)

    # ---- packed param blobs ----
    blob0 = np.zeros((128, 2929), np.float16)
    blob0[:, 0:129] = W1[0:128]
    blob0[:, 129:258] = W1[128:256]
    blob0[0:2, 258:387] = W1[256:258]
    for k in range(4):
        blob0[:, 387 + k * 129 : 387 + (k + 1) * 129] = W1[258 + 128 * k : 258 + 128 * (k + 1)]
    blob0[:, 903:967] = W2[0:128]
    blob0[0:1, 967:1031] = W2[128:129]
    for k in range(2):
        blob0[:, 1031 + k * 64 : 1031 + (k + 1) * 64] = W2[129 + 128 * k : 129 + 128 * (k + 1)]
    blob0[0:64, 1159:1193] = W3[0:64]
    blob0[:, 1193:1227] = W3[64:192]
    blob0[0:64, 2763:2797] = W3[0:64]
    blob0[64:65, 2763:2797] = b3.reshape(1, 34)
    blob0[0:64, 2797 + 32 : 2863] = W3[0:64]
    blob0[64:65, 2797 + 32 : 2863] = b3.reshape(1, 34)
    blobf = np.zeros((128, 200), np.float32)
    blobf[:, 0:2] = np.stack([g1[0:128], be1[0:128]], 1)
    blobf[0:1, 2:4] = np.stack([g1[128:129], be1[128:129]], 1)
    blobf[0:64, 4:6] = np.stack([g2, be2], 1)
    blobf[0:34, 7:8] = b3.reshape(34, 1)
    blobf[64:98, 7:8] = b3.reshape(34, 1)
    eye = np.eye(64, dtype=np.float32)
    blobf[0:64, 8:72] = eye
    blobf[64:128, 8:72] = eye
    blobf[0:64, 72:136] = eye
    blobf[0:64, 136:200] = eye

    featsTf = np.ascontiguousarray(feats.T)
    s3Tf = skip3.T
    s2Tf = skip2.T
    s1Tf = skip1.T

    in_maps = []
    for c in range(NCORES):
        pts1, slots1 = p1_sorted[c]
        s3T = np.zeros((512, n1p), np.float16)
        s3T[:, slots1] = s3Tf[:, pts1]
        P1m = np.zeros((128, n1p), np.float16)
        P1m[(idx1[pts1] - C3 * c) % 128, slots1] = 1

        pts2 = P2s[c]
        sl2 = slot2[pts2]
        s2T = np.zeros((257, n2p), np.float16)
        s2T[0:256, sl2] = s2Tf[:, pts2]
        s2T[256, :] = 1
        P2m = np.zeros((128, n2p), np.float16)
        P2m[srcslot2[pts2] % 128, sl2] = 1

        omap = outmaps[c]
        valid = omap >= 0
        s1T = np.zeros((128, n3p), np.float16)
        s1T[:, valid] = s1Tf[:, omap[valid]]

        blob = blob0.copy()
        fc = featsTf[:, C3 * c : C3 * (c + 1)].astype(np.float16)
        blob[:, 1227:1739] = fc[0:128]
        blob[:, 1739:2251] = fc[128:256]
        blob[0:2, 2251:2763] = fc[256:258]
        in_maps.append(
            {
                "s3T": s3T,
                "s2T": s2T,
                "s1T": s1T,
                "P1": P1m,
                "P2": P2m,
                "blob": blob,
                "blobf": blobf,
            }
        )

    return key, in_maps, outmaps


def _install_ntff_hook():
    import sys as _sys
    import types

    if "antenv.axon_hooks" in _sys.modules:
        return
    mod = types.ModuleType("antenv.axon_hooks")
    holder = {}
    mod.set_axon_ntff_profile_hook = lambda h: holder.__setitem__("h", h)
    mod.get_axon_ntff_profile_hook = lambda: holder.get("h")
    _sys.modules["antenv.axon_hooks"] = mod
    try:
        from trn_agent_boot.trn_boot import _ntff_profile_via_ctypes

        h = _ntff_profile_via_ctypes("/opt/axon/libaxon_pjrt.so")
        if h is not None:
            holder["h"] = h
    except Exception:
        pass


def kernel(_want_trace=False, **inputs):
    if _want_trace:
        _install_ntff_hook()
    key, in_maps, outmaps = prepare(**inputs)
    nc = _get_program(key, *key)

    res = bass_utils.run_bass_kernel_spmd(
        nc, in_maps, core_ids=list(range(NCORES)), trace=_want_trace
    )

    n1p, B1, n2p, cells2, Mtuple, KMAX = key
    _, _, _, n3p, s1chunks = _cls_layout(list(Mtuple), KMAX)
    chunk_ops, PK = _pair_ops(s1chunks)
    out = np.empty((N0, 34), np.float32)
    for c in range(NCORES):
        r = res.results[c]["out"]  # [98, PK]
        o34 = np.empty((34, n3p), np.float32)
        for ops in chunk_ops:
            for tag, k, ck, g0, s0, po in ops:
                o34[:, g0 : g0 + ck] = r[0:34, po : po + ck]
                if tag == "pair":
                    o34[:, g0 + ck : g0 + 2 * ck] = r[64:98, po : po + ck]
        omap = outmaps[c]
        valid = omap >= 0
        out[omap[valid]] = o34.T[valid]

    if _want_trace:
        kernel._last_trace = res
    return out
